# revision 56
# baseline (speedup 1.0000x reference)
"""Trainium2 Bass kernel for nn_DeepSetLayer (GNN attention message passing).

Design (8 NeuronCores, graph-parallel by destination node). All heavy
matmuls use fp8 DoubleRow(SwInterleave) at 0.5 cyc/col; per-matmul cost
on TRN2 is dominated by the inline weight load (~300 GB/s), so the
layout is chosen to minimize bytes entering the PE as stationary
operands.

  Host (pure layout + dtype casts; only weight-sized float math):
    bin-pack dst nodes into 8 cores x 50 blocks (<=128 dsts, <=13x128
    edges). Within a block, PAIR-ALIGN edges of each dst: tiles 2c/2c+1
    hold the two halves of pair column c (same dst per lane), tile 12
    holds odd singles. Streams:
      XQK fp8 [feat, block, 10, 128, 2]: SwInterleave weight chunks
        (q-pairs / k-per-pair-column shared / single q|k). Sharing the
        dst features across each pair column cuts qk weight bytes 25%.
      XEA fp8 [edge, plane, 129]: src feats + ones col (agg moving side;
        col 128 accumulates seg for free).
    Pad slots carry pinv-probe features that force exp(score) to round
    to exactly 0.0 in fp8 (weight-only pinv math).
  Device per block (5-deep software pipeline, PE queue never waits):
    q/k: 6+3+1 fp8 DRI matmuls + one fp8-DR bias matmul -> [128e, 240]
    scores = sum(tanh(q)*k)/sqrt(S) (ACT tanh + 2 DVE mults/reduces)
    exp -> fp8 bytes; ONE gpsimd local_scatter writes int16 exp-pairs
      into S_w (reversed columns: SwInterleave weight order)
    att[128d, 129] = 7 DRI matmuls (S_w pairs x XEA pairs), dst-major;
      col 128 = seg
    epilogue: rec=1/seg (DVE); attn = att*rec (ACT); PE-transpose;
      xpre = attnT.T@w2T + nshT.T@w1T (one psum acc) + b2 (DVE add)
  Per group of 5 blocks: rsqrt (DVE Newton bit-trick), relu(xpre*rin)
  -> bf16 out DMA'd while the main loop continues (no serial tail).
  Host: inverse-permute per-core outputs into the full [N, F] f32.
"""

import math
import sys

sys.path.insert(0, "/opt/trn_rl_repo")

import heapq

import ml_dtypes
import numpy as np

import concourse.bacc as bacc
import concourse.bass as bass
import concourse.mybir as mybir
import concourse.tile as tile
from concourse.bass_utils import run_bass_kernel_spmd

N = 50000
E = 600000
F = 128
S = 12
NCORES = 8
B = 50            # blocks per core
TB = 13           # 128-edge tiles per block
DSTS = B * 128    # 6400 padded dst slots per core
G = 5             # blocks per DMA group
NG = B // G
NPL = B * TB      # total edge tiles (planes) per core
GPL = G * TB      # planes per group
INV_SQRT_S = 1.0 / math.sqrt(float(S))

# Aggregation S_w mode:
#  "hilo": 13 DRI matmuls/block, S_w split hi+lo fp8 (bf16-grade accuracy)
#  "pair": host pair-aligns edges of each dst into 6 tile-pair columns +
#          1 single tile; 6 DRI + 1 regular matmul/block, single fp8 S_w
#          (faster: half the scatter elems + half the agg weight loads)
AGG_MODE = "pair"
NPAIR = 6            # pair columns in pair mode (tiles 0..11), tile 12 single
if AGG_MODE == "hilo":
    SWE = TB * 128 + 4   # sw int16 elems per block
    NIDX = 14
    EWB = 28             # expw bytes per block
else:
    SWE = (NPAIR + 1) * 128 + 4
    NIDX = 8
    EWB = 16

f32 = mybir.dt.float32
bf16 = mybir.dt.bfloat16
i16 = mybir.dt.int16
i32 = mybir.dt.int32
fp8 = mybir.dt.float8e4
bf16_np = ml_dtypes.bfloat16
fp8_np = ml_dtypes.float8_e4m3fn

DR = mybir.MatmulPerfMode.DoubleRow
DRI = mybir.MatmulPerfMode.DoubleRowSwInterleave

_compiled = {}


def _pack_bins(deg):
    """Assign each dst to one of NCORES*B bins (<=128 dsts, <=TB*128 edges),
    balancing edge counts."""
    nbins = NCORES * B
    order = np.argsort(-deg, kind="stable")
    b_e = np.zeros(nbins, np.int64)
    b_n = np.zeros(nbins, np.int64)
    bins_dsts = [[] for _ in range(nbins)]
    heap = [(0, b) for b in range(nbins)]
    heapq.heapify(heap)
    for dst in order:
        dst = int(dst)
        d = int(deg[dst])
        stash = []
        while True:
            ec, b = heapq.heappop(heap)
            if ec != b_e[b]:
                continue
            if b_n[b] < 128 and b_e[b] + d <= TB * 128:
                break
            stash.append((ec, b))
        bins_dsts[b].append(dst)
        b_e[b] += d
        b_n[b] += 1
        if b_n[b] < 128:
            heapq.heappush(heap, (int(b_e[b]), b))
        for it in stash:
            heapq.heappush(heap, it)
    return bins_dsts


def _pad_probes(Wq, bq, Wk, bk):
    """Pad-slot feature vectors (from weights only) that force the pad
    score so negative that exp() rounds to exactly 0 in fp8."""
    Wq = np.asarray(Wq, np.float64)
    Wk = np.asarray(Wk, np.float64)
    bq = np.asarray(bq, np.float64)
    bk = np.asarray(bk, np.float64)
    x_padQ = np.linalg.pinv(Wq) @ (np.arctanh(0.9) * np.ones(S) - bq)
    x_padK = np.linalg.pinv(Wk) @ (-4.8 * np.ones(S) - bk)
    q8 = lambda v: np.asarray(v, fp8_np).astype(np.float64)
    qt = np.tanh(q8(x_padQ) @ q8(Wq.T) + bq)
    kt = q8(x_padK) @ q8(Wk.T) + bk
    sc = float(np.sum(qt * kt))
    assert sc < -40.0, f"pad probe score too high: {sc}"
    return x_padQ.astype(np.float32), x_padK.astype(np.float32)


def _host_prep(node_data, src, dst, Wq, bq, Wk, bk):
    x = np.ascontiguousarray(np.asarray(node_data, np.float32))
    loops = np.arange(N, dtype=np.int64)
    s_all = np.concatenate([np.asarray(src, np.int64), loops])
    d_all = np.concatenate([np.asarray(dst, np.int64), loops])

    deg = np.bincount(d_all, minlength=N)
    bins_dsts = _pack_bins(deg)

    perm = np.full(NCORES * DSTS, -1, dtype=np.int64)
    for b, dlist in enumerate(bins_dsts):
        core, blk = divmod(b, B)
        base = core * DSTS + blk * 128
        perm[base : base + len(dlist)] = dlist

    # CSR of edges by dst
    eorder = np.argsort(d_all, kind="stable")
    indptr = np.zeros(N + 1, dtype=np.int64)
    np.cumsum(deg, out=indptr[1:])
    s_sorted = s_all[eorder]

    if AGG_MODE == "pair":
        x_padQ, x_padK = _pad_probes(Wq, bq, Wk, bk)
    else:
        x_padQ = x_padK = np.zeros(F, np.float32)

    # extended tables: row N = pad probe (Q for srcs, K for dsts), fp8
    nodeQ = np.vstack([x, x_padQ[None, :]]).astype(fp8_np)
    nodeK = np.vstack([x, x_padK[None, :]]).astype(fp8_np)
    nodeE = np.vstack([x, np.zeros((1, F), np.float32)]).astype(fp8_np)
    nsh_bf = np.ascontiguousarray(x.astype(bf16_np))

    per_core = []
    for core in range(NCORES):
        srcmat = np.full((128, NPL), N, np.int64)   # N -> pad row
        dstmat = np.full((128, NPL), N, np.int64)
        dstloc = np.full((128, NPL), -1, dtype=np.int64)

        for blk in range(B):
            dlist = bins_dsts[core * B + blk]
            pl0 = blk * TB
            if AGG_MODE == "hilo":
                ss, dd_, dl = [], [], []
                for j, d0 in enumerate(dlist):
                    es = s_sorted[indptr[d0] : indptr[d0 + 1]]
                    ss.append(es)
                    dd_.append(np.full(len(es), d0, np.int64))
                    dl.append(np.full(len(es), j, np.int64))
                ss = np.concatenate(ss) if ss else np.zeros(0, np.int64)
                dd_ = np.concatenate(dd_) if dd_ else np.zeros(0, np.int64)
                dl = np.concatenate(dl) if dl else np.zeros(0, np.int64)
                ne = len(ss)
                assert ne <= TB * 128, f"block overflow {ne}"
                sp = np.full(TB * 128, N, np.int64)
                sp[:ne] = ss
                dp = np.full(TB * 128, N, np.int64)
                dp[:ne] = dd_
                lp = np.full(TB * 128, -1, np.int64)
                lp[:ne] = dl
                srcmat[:, pl0 : pl0 + TB] = sp.reshape(TB, 128).T
                dstmat[:, pl0 : pl0 + TB] = dp.reshape(TB, 128).T
                dstloc[:, pl0 : pl0 + TB] = lp.reshape(TB, 128).T
            else:
                # pair mode: tiles 2c/2c+1 are the two DR planes of pair
                # column c (same dst in both halves per lane); tile 12
                # holds odd singles.
                pairs, singles = [], []
                for j, d0 in enumerate(dlist):
                    es = s_sorted[indptr[d0] : indptr[d0 + 1]]
                    npair = len(es) // 2
                    for i in range(npair):
                        pairs.append((j, d0, es[2 * i], es[2 * i + 1]))
                    if len(es) % 2:
                        singles.append((j, d0, es[-1]))
                while len(pairs) > NPAIR * 128:
                    j, d0, a, bsrc = pairs.pop()
                    singles.append((j, d0, a))
                    singles.append((j, d0, bsrc))
                assert len(singles) <= 128, f"singles overflow {len(singles)}"
                for s_i, (j, d0, a, bsrc) in enumerate(pairs):
                    c, lane = divmod(s_i, 128)
                    srcmat[lane, pl0 + 2 * c] = a
                    srcmat[lane, pl0 + 2 * c + 1] = bsrc
                    dstmat[lane, pl0 + 2 * c] = d0
                    dstmat[lane, pl0 + 2 * c + 1] = d0
                    dstloc[lane, pl0 + 2 * c] = j
                    dstloc[lane, pl0 + 2 * c + 1] = j
                for s_i, (j, d0, a) in enumerate(singles):
                    srcmat[s_i, pl0 + 2 * NPAIR] = a
                    dstmat[s_i, pl0 + 2 * NPAIR] = d0
                    dstloc[s_i, pl0 + 2 * NPAIR] = j

        # local_scatter indices [128, B, NIDX] int16 (pad -> -1: skipped)
        idx16 = np.full((128, B, NIDX), -1, np.int16)
        for blk in range(B):
            dl_blk = dstloc[:, blk * TB : (blk + 1) * TB]  # [128, TB]
            if AGG_MODE == "hilo":
                # one int16 (hi,lo) per edge; SwInterleave reads columns
                # reversed -> 127 - dstloc
                idxv = (
                    np.arange(TB, dtype=np.int64)[None, :] * 128
                    + (127 - dl_blk)
                )
                idxv[dl_blk < 0] = -1
                idx16[:, blk, :TB] = idxv.astype(np.int16)
            else:
                # pair cols: one int16 (expA,expB) per pair, reversed col
                for c in range(NPAIR):
                    dlc = dl_blk[:, 2 * c]
                    dlc2 = dl_blk[:, 2 * c + 1]
                    v = c * 128 + (127 - np.where(dlc >= 0, dlc, dlc2))
                    v[(dlc < 0) & (dlc2 < 0)] = -1
                    idx16[:, blk, c] = v.astype(np.int16)
                # single tile: (fp8 exp, 0x00) int16, reversed column
                dls = dl_blk[:, 2 * NPAIR]
                v = NPAIR * 128 + (127 - dls)
                v[dls < 0] = -1
                idx16[:, blk, NPAIR] = v.astype(np.int16)
        idx16 = np.ascontiguousarray(idx16.reshape(128, B * NIDX))

        # edge-ordered feature streams
        g_src = nodeQ[srcmat]                   # [128 e, NPL, F] fp8
        g_dst = nodeK[dstmat]                   # [128 e, NPL, F] fp8
        # XQK: 10 SwInterleave weight chunks per block (columns reversed):
        #   c=0..5: (XTG_2c, XTG_2c+1)        -> q of both pair halves
        #   c=6..8: (XDT_col2d, XDT_col2d+1)  -> k of two pair columns
        #   c=9   : (XTG_12, XDT_12)          -> single tile q|k
        xq = g_src.transpose(2, 1, 0)[:, :, ::-1]   # [F, NPL, 128] reversed
        xd = g_dst.transpose(2, 1, 0)[:, :, ::-1]
        xqk = np.empty((F, B, 10, 128, 2), fp8_np)
        for blk in range(B):
            pl0 = blk * TB
            for c in range(NPAIR):
                xqk[:, blk, c, :, 0] = xq[:, pl0 + 2 * c]
                xqk[:, blk, c, :, 1] = xq[:, pl0 + 2 * c + 1]
            for d in range(NPAIR // 2):
                xqk[:, blk, NPAIR + d, :, 0] = xd[:, pl0 + 4 * d]
                xqk[:, blk, NPAIR + d, :, 1] = xd[:, pl0 + 4 * d + 2]
            xqk[:, blk, 9, :, 0] = xq[:, pl0 + 2 * NPAIR]
            xqk[:, blk, 9, :, 1] = xd[:, pl0 + 2 * NPAIR]
        xqk = np.ascontiguousarray(xqk.reshape(F, B * 2560))
        # XEA: [128e, NPL, 129] fp8 (src feats + ones col)
        g_ea = nodeE[srcmat]
        xea = np.empty((128, NPL, F + 1), fp8_np)
        xea[:, :, :F] = g_ea
        xea[:, :, F] = fp8_np(1.0)
        xea = np.ascontiguousarray(xea.reshape(128, NPL * (F + 1)))

        nshT = np.zeros((F, DSTS), bf16_np)
        sl = perm[core * DSTS : (core + 1) * DSTS]
        valid = sl >= 0
        nshT[:, valid] = nsh_bf[sl[valid]].T

        per_core.append(dict(xqk=xqk, xea=xea, nshT=nshT, idx16=idx16))

    return per_core, perm


def _build_nc():
    nc = bacc.Bacc(
        "TRN2",
        target_bir_lowering=False,
        debug=False,
        enable_asserts=False,
        num_devices=NCORES,
    )
    AF = mybir.ActivationFunctionType
    OP = mybir.AluOpType

    xqk_d = nc.dram_tensor("xqk", [F, B * 2560], fp8, kind="ExternalInput")
    xea_d = nc.dram_tensor("xea", [128, NPL * (F + 1)], fp8, kind="ExternalInput")
    nshT_d = nc.dram_tensor("nshT", [F, DSTS], bf16, kind="ExternalInput")
    idx16_d = nc.dram_tensor("idx16", [128, B * NIDX], i16, kind="ExternalInput")
    wqk2_d = nc.dram_tensor("wqk2", [F, 3 * 48], fp8, kind="ExternalInput")
    w1T_d = nc.dram_tensor("w1T", [F, F], bf16, kind="ExternalInput")
    w2T_d = nc.dram_tensor("w2T", [F, F], bf16, kind="ExternalInput")
    bqkr_d = nc.dram_tensor("bqkr", [128, 240], bf16, kind="ExternalInput")
    b2t_d = nc.dram_tensor("b2t", [128, F], bf16, kind="ExternalInput")
    ident_d = nc.dram_tensor("ident", [128, 128], bf16, kind="ExternalInput")
    out_d = nc.dram_tensor("out", [DSTS, F], bf16, kind="ExternalOutput")

    with tile.TileContext(nc) as tc:
        with tc.tile_pool(name="const", bufs=1) as const:
            # score-path consts first: block 0 needs them immediately
            wqk3 = const.tile([F, 3, 2, 24], fp8)
            nc.sync.dma_start(
                wqk3[:].rearrange("p w a b -> p (w a b)"), wqk2_d[:]
            )
            wq2 = wqk3[:, 0, :, :]
            wk2 = wqk3[:, 1, :, :]
            wqks = wqk3[:, 2, :, :]
            # q/k bias pre-tiled across partitions; added on DVE (not PE)
            bqkr = const.tile([128, 240], bf16)
            nc.sync.dma_start(bqkr[:], bqkr_d[:])
            idx16 = const.tile([128, B * NIDX], i16)
            # epilogue-path consts issued on the scalar ring (needed later)
            nshT = const.tile([F, DSTS], bf16)
            w1T = const.tile([F, F], bf16)
            w2T = const.tile([F, F], bf16)
            b2t = const.tile([128, F], bf16)
            ident = const.tile([128, 128], bf16)
            ones_row = const.tile([1, 128], bf16)
            nc.vector.memset(ones_row[:], 1.0)
            # bf16 xpre for all blocks, plus row sum-of-squares
            xpre_all = const.tile([128, B, F], bf16)
            ssq_all = const.tile([128, B], f32)
            # exp bytes per block (mode-dependent layout); int16 view
            # feeds the scatter
            expw_all = const.tile([128, B, EWB], fp8)
            nc.gpsimd.memset(expw_all[:], 0.0)

            with (
                tc.tile_pool(name="xqkp", bufs=3) as xqkp,
                tc.tile_pool(name="xeap", bufs=4) as xeap,
                tc.tile_pool(name="wk3", bufs=3) as wk3,
                tc.tile_pool(name="wk4", bufs=4) as wk4,
                tc.tile_pool(name="swp", bufs=5) as swp,
                tc.tile_pool(name="outp", bufs=2) as outp,
                tc.tile_pool(name="p2", bufs=2) as p2,
                tc.tile_pool(name="ps_qk", bufs=2, space="PSUM") as ps_qk,
                tc.tile_pool(name="ps_att", bufs=2, space="PSUM") as ps_att,
                tc.tile_pool(name="ps_T", bufs=2, space="PSUM") as ps_T,
                tc.tile_pool(name="ps_x", bufs=2, space="PSUM") as ps_x,
            ):
                xqk_tiles = {}
                xea_tiles = {}

                def dma_group(g, nchunk=2):
                    if g >= NG:
                        return
                    q0 = g * G * 2560
                    XQK = xqkp.tile([128, G * 2560], fp8, tag="XQK")
                    cs = G * 2560 // nchunk
                    for c in range(nchunk):
                        nc.sync.dma_start(
                            XQK[:, c * cs : (c + 1) * cs],
                            xqk_d[:, q0 + c * cs : q0 + (c + 1) * cs],
                        )
                    e0 = g * GPL * 129
                    XEA = xeap.tile([128, GPL * 129], fp8, tag="XEA")
                    es = GPL * 129 // max(nchunk // 2, 1)
                    for c in range(max(nchunk // 2, 1)):
                        nc.sync.dma_start(
                            XEA[:, c * es : (c + 1) * es],
                            xea_d[:, e0 + c * es : e0 + (c + 1) * es],
                        )
                    xqk_tiles[g] = XQK
                    xea_tiles[g] = XEA

                def dma_consts():
                    # all input DMAs ride the SP (sync) ring: its sequencer
                    # has no compute ops, so ring-capacity stalls are free.
                    # The ACT ring carries compute only (DGE trigger stalls
                    # there were delaying the first tanh by ~20us).
                    nc.sync.dma_start(ident[:], ident_d[:])
                    nc.sync.dma_start(w2T[:], w2T_d[:])
                    nc.sync.dma_start(w1T[:], w1T_d[:])
                    nc.sync.dma_start(b2t[:], b2t_d[:])
                    nc.sync.dma_start(nshT[:], nshT_d[:])

                # pipeline state carried between stages
                sw_of = {}
                attps_of = {}
                rec_of = {}
                attn_of = {}
                attnT_of = {}

                def stage_score(b):
                    # psqk layout [128, 240]:
                    #   [0:144)   q of pair tiles 0..11 (6 DRIs x 24)
                    #   [144:156) q single tile; [156:168) k single tile
                    #   [168:240) k of pair cols 0..5 (3 DRIs x 24)
                    g, bb = divmod(b, G)
                    XQK = xqk_tiles[g]
                    boff = bb * 2560
                    psqk = ps_qk.tile([128, 240], f32, tag="psqk")
                    for c in range(NPAIR):
                        lhs = XQK[
                            :, boff + c * 256 : boff + (c + 1) * 256
                        ].rearrange("p (e a) -> p a e", a=2)
                        nc.tensor.matmul(
                            psqk[:, c * 24 : (c + 1) * 24], lhs, wq2,
                            start=(c == 0), stop=False, perf_mode=DRI,
                        )
                    for d in range(NPAIR // 2):
                        lhs = XQK[
                            :,
                            boff + (NPAIR + d) * 256 : boff
                            + (NPAIR + d + 1) * 256,
                        ].rearrange("p (e a) -> p a e", a=2)
                        nc.tensor.matmul(
                            psqk[:, 168 + d * 24 : 168 + (d + 1) * 24],
                            lhs, wk2,
                            start=False, stop=False, perf_mode=DRI,
                        )
                    lhs = XQK[:, boff + 9 * 256 : boff + 10 * 256].rearrange(
                        "p (e a) -> p a e", a=2
                    )
                    nc.tensor.matmul(
                        psqk[:, 144:168], lhs, wqks,
                        start=False, stop=True, perf_mode=DRI,
                    )
                    # bias add on DVE (bqkr pre-tiled across partitions)
                    qk_sb = wk3.tile([128, 240], bf16, tag="qk_sb")
                    nc.vector.tensor_tensor(
                        qk_sb[:], psqk[:], bqkr[:], OP.add
                    )
                    q_sb = wk3.tile([128, 156], bf16, tag="qsb")
                    nc.scalar.activation(q_sb[:], qk_sb[:, 0:156], AF.Tanh)
                    prod = wk3.tile([128, NPAIR, 2, S], bf16, tag="prod")
                    kbc = (
                        qk_sb[:, 168:240]
                        .rearrange("p (c s) -> p c s", c=NPAIR)
                        .unsqueeze(2)
                        .broadcast_to([128, NPAIR, 2, S])
                    )
                    nc.vector.tensor_tensor(
                        prod[:],
                        q_sb[:, 0:144].rearrange(
                            "p (c a s) -> p c a s", c=NPAIR, a=2
                        ),
                        kbc, OP.mult,
                    )
                    prods = wk3.tile([128, S], bf16, tag="prods")
                    nc.vector.tensor_tensor(
                        prods[:], q_sb[:, 144:156], qk_sb[:, 156:168],
                        OP.mult,
                    )
                    scores = wk3.tile([128, TB], f32, tag="scores")
                    nc.vector.tensor_reduce(
                        scores[:, 0 : 2 * NPAIR].rearrange(
                            "p (c a) -> p c a", c=NPAIR
                        ),
                        prod[:], mybir.AxisListType.X, OP.add,
                    )
                    nc.vector.tensor_reduce(
                        scores[:, 2 * NPAIR : TB], prods[:],
                        mybir.AxisListType.X, OP.add,
                    )
                    ew = expw_all[:, b, :]
                    if AGG_MODE == "hilo":
                        exps = wk3.tile([128, TB], bf16, tag="exps")
                        nc.scalar.activation(
                            exps[:], scores[:], AF.Exp, scale=INV_SQRT_S
                        )
                        # hi fp8 at even bytes, lo = exp - hi at odd bytes
                        ew2 = ew.rearrange("p (t a) -> p t a", a=2)
                        hi = ew2[:, 0:TB, 0]
                        lo_ = ew2[:, 0:TB, 1]
                        nc.scalar.activation(hi, exps[:], AF.Copy)
                        nc.vector.tensor_tensor(lo_, exps[:], hi, OP.subtract)
                    else:
                        # exps fp8 at bytes 0..12 (pairs + single; byte 13
                        # stays 0 from the one-time memset -> dead plane)
                        nc.scalar.activation(
                            ew[:, 0 : TB], scores[:],
                            AF.Exp, scale=INV_SQRT_S,
                        )
                    sw = swp.tile([128, SWE], i16, tag="sw")
                    nc.gpsimd.local_scatter(
                        sw[:],
                        ew.bitcast(i16),
                        idx16[:, b * NIDX : (b + 1) * NIDX],
                        channels=128,
                        num_elems=SWE,
                        num_idxs=NIDX,
                    )
                    sw_of[b] = sw

                def stage_agg(b):
                    g, bb = divmod(b, G)
                    XEA = xea_tiles[g]
                    sw8 = sw_of[b][:].bitcast(fp8)
                    att = ps_att.tile([128, F + 1], f32, tag="att")
                    if AGG_MODE == "hilo":
                        for t in range(TB):
                            lhs = sw8[:, t * 256 : (t + 1) * 256].rearrange(
                                "p (e a) -> p a e", a=2
                            )
                            eo = (bb * TB + t) * 129
                            rhs = (
                                XEA[:, eo : eo + 129]
                                .unsqueeze(1)
                                .broadcast_to([128, 2, 129])
                            )
                            nc.tensor.matmul(
                                att[:], lhs, rhs,
                                start=(t == 0), stop=(t == TB - 1),
                                perf_mode=DRI,
                            )
                    else:
                        for c in range(NPAIR):
                            lhs = sw8[:, c * 256 : (c + 1) * 256].rearrange(
                                "p (e a) -> p a e", a=2
                            )
                            eo = (bb * TB + 2 * c) * 129
                            rhs = XEA[:, eo : eo + 258].rearrange(
                                "p (a e) -> p a e", a=2
                            )
                            nc.tensor.matmul(
                                att[:], lhs, rhs,
                                start=(c == 0), stop=False, perf_mode=DRI,
                            )
                        # single tile: DRI with a zero second plane
                        lhs1 = sw8[
                            :, NPAIR * 256 : (NPAIR + 1) * 256
                        ].rearrange("p (e a) -> p a e", a=2)
                        eo = (bb * TB + 2 * NPAIR) * 129
                        rhs1 = (
                            XEA[:, eo : eo + 129]
                            .unsqueeze(1)
                            .broadcast_to([128, 2, 129])
                        )
                        nc.tensor.matmul(
                            att[:], lhs1, rhs1,
                            start=False, stop=True, perf_mode=DRI,
                        )
                    attps_of[b] = att
                    del sw_of[b]

                def stage_epiA(b):
                    att = attps_of[b]
                    rec = wk4.tile([128, 1], f32, tag="rec")
                    nc.vector.reciprocal(rec[:], att[:, F : F + 1])
                    attn = wk4.tile([128, F], bf16, tag="attn")
                    nc.scalar.activation(
                        attn[:], att[:, 0:F], AF.Copy, scale=rec[:]
                    )
                    attn_of[b] = attn
                    del attps_of[b]

                def stage_epiB1(b):
                    attn = attn_of[b]
                    tps = ps_T.tile([128, 128], bf16, tag="tps")
                    nc.tensor.matmul(
                        tps[:], attn[:], ident[:], is_transpose=True
                    )
                    attnT = wk4.tile([128, 128], bf16, tag="attnT")
                    if b % 2 == 0:
                        nc.vector.tensor_copy(attnT[:], tps[:])
                    else:
                        nc.scalar.activation(attnT[:], tps[:], AF.Copy)
                    attnT_of[b] = attnT
                    del attn_of[b]

                def stage_epiB2(b):
                    attnT = attnT_of[b]
                    px = ps_x.tile([128, F], f32, tag="px")
                    nc.tensor.matmul(
                        px[:], attnT[:], w2T[:], start=True, stop=False
                    )
                    nc.tensor.matmul(
                        px[:], nshT[:, b * 128 : (b + 1) * 128], w1T[:],
                        start=False, stop=True,
                    )
                    # xpre = px + b2 (b2 pre-tiled across partitions)
                    nc.vector.tensor_tensor(
                        xpre_all[:, b, :], px[:], b2t[:], OP.add
                    )
                    sqd = wk4.tile([128, F], bf16, tag="sqd")
                    nc.vector.scalar_tensor_tensor(
                        sqd[:], xpre_all[:, b, :], 1.0, xpre_all[:, b, :],
                        OP.mult, OP.mult,
                        accum_out=ssq_all[:, b : b + 1],
                    )
                    del attnT_of[b]

                def stage_p2(g):
                    # rsqrt via bit-trick seed + 2 Newton steps (table-free)
                    ssq = ssq_all[:, g * G : (g + 1) * G]
                    yt = p2.tile([128, G], i32, tag="yt")
                    nc.vector.tensor_scalar(
                        yt[:], ssq.bitcast(i32), 1, None,
                        OP.logical_shift_right,
                    )
                    y0 = p2.tile([128, G], i32, tag="y0")
                    nc.vector.tensor_scalar(
                        y0[:], yt[:], -1, 0x5F3759DF, OP.mult, OP.add
                    )
                    h = p2.tile([128, G], f32, tag="h")
                    nc.vector.tensor_scalar(h[:], ssq, -0.5, None, OP.mult)
                    y = y0[:].bitcast(f32)
                    for _ in range(2):
                        y2 = p2.tile([128, G], f32, tag="y2")
                        nc.vector.tensor_tensor(y2[:], y, y, OP.mult)
                        t2 = p2.tile([128, G], f32, tag="t2")
                        nc.vector.tensor_tensor(t2[:], y2[:], h[:], OP.mult)
                        t3 = p2.tile([128, G], f32, tag="t3")
                        nc.vector.tensor_scalar(t3[:], t2[:], 1.5, None, OP.add)
                        yn = p2.tile([128, G], f32, tag="yn")
                        nc.vector.tensor_tensor(yn[:], y, t3[:], OP.mult)
                        y = yn[:]
                    ot = outp.tile([128, G, F], bf16, tag="ot")
                    for bb in range(G):
                        b = g * G + bb
                        if bb % 2 == 0:
                            nc.scalar.activation(
                                ot[:, bb, :], xpre_all[:, b, :], AF.Relu,
                                scale=yn[:, bb : bb + 1],
                            )
                        else:
                            nc.vector.tensor_scalar(
                                ot[:, bb, :], xpre_all[:, b, :],
                                yn[:, bb : bb + 1], 0.0, OP.mult, OP.max,
                            )
                    nc.sync.dma_start(
                        out_d[g * G * 128 : (g + 1) * G * 128, :].rearrange(
                            "(j p) f -> p j f", p=128
                        ),
                        ot[:],
                    )

                # ---- main pipelined loop ----
                dma_group(0)
                nc.sync.dma_start(idx16[:], idx16_d[:])
                dma_consts()
                dma_group(1)
                dma_group(2)
                for i in range(B + 6):
                    if i < B and i % G == 0:
                        if i > 0:
                            dma_group(i // G + 2)
                    if i < B:
                        stage_score(i)
                    if 2 <= i <= B + 1:
                        stage_agg(i - 2)
                    if 3 <= i <= B + 2:
                        stage_epiA(i - 3)
                    if 4 <= i <= B + 3:
                        stage_epiB1(i - 4)
                    if 5 <= i <= B + 4:
                        stage_epiB2(i - 5)
                    j = i - 5  # epiB2 just finished block j
                    if j >= 0 and (j + 1) % G == 0:
                        stage_p2(j // G)

    nc.compile()
    return nc


def get_nc():
    if "nc" not in _compiled:
        _compiled["nc"] = _build_nc()
    return _compiled["nc"]


def _make_in_maps(node_data, src, dst, Wq, bq, Wk, bk, W1, W2, b2):
    per_core, perm = _host_prep(node_data, src, dst, Wq, bq, Wk, bk)
    wq8 = np.asarray(Wq, np.float32).T.astype(fp8_np)  # [F, S]
    wk8 = np.asarray(Wk, np.float32).T.astype(fp8_np)
    wqk2 = np.zeros((F, 3, 2, 24), fp8_np)
    wqk2[:, 0, 0, 0:S] = wq8       # q-pair DRI: (q | q)
    wqk2[:, 0, 1, S : 2 * S] = wq8
    wqk2[:, 1, 0, 0:S] = wk8       # k-pair DRI: (k | k)
    wqk2[:, 1, 1, S : 2 * S] = wk8
    wqk2[:, 2, 0, 0:S] = wq8       # single DRI: (q | k)
    wqk2[:, 2, 1, S : 2 * S] = wk8
    bias240 = np.zeros(240, np.float32)
    bias240[0:144] = np.tile(np.asarray(bq, np.float32), 2 * NPAIR)
    bias240[144:156] = np.asarray(bq, np.float32)
    bias240[156:168] = np.asarray(bk, np.float32)
    bias240[168:240] = np.tile(np.asarray(bk, np.float32), NPAIR)
    consts = dict(
        wqk2=np.ascontiguousarray(wqk2.reshape(F, 144)),
        w1T=np.ascontiguousarray(np.asarray(W1, np.float32).T).astype(bf16_np),
        w2T=np.ascontiguousarray(np.asarray(W2, np.float32).T).astype(bf16_np),
        bqkr=np.ascontiguousarray(
            np.tile(bias240[None, :], (128, 1))
        ).astype(bf16_np),
        b2t=np.ascontiguousarray(
            np.tile(np.asarray(b2, np.float32)[None, :], (128, 1))
        ).astype(bf16_np),
        ident=np.eye(128, dtype=bf16_np),
    )
    in_maps = []
    for core in range(NCORES):
        m = dict(consts)
        m.update(per_core[core])
        in_maps.append(m)
    return in_maps, perm


def run(node_data, src, dst, Wq, bq, Wk, bk, W1, W2, b2, trace=False):
    in_maps, perm = _make_in_maps(
        node_data, src, dst, Wq, bq, Wk, bk, W1, W2, b2
    )
    nc = get_nc()
    res = run_bass_kernel_spmd(nc, in_maps, list(range(NCORES)), trace=trace)
    out = np.zeros((N, F), dtype=np.float32)
    for core in range(NCORES):
        o = np.asarray(res.results[core]["out"]).astype(np.float32)
        sl = perm[core * DSTS : (core + 1) * DSTS]
        valid = sl >= 0
        out[sl[valid]] = o[valid]
    return out, res


def kernel(node_data, src, dst, Wq, bq, Wk, bk, W1, W2, b2):
    out, _ = run(node_data, src, dst, Wq, bq, Wk, bk, W1, W2, b2, trace=False)
    return out


if __name__ == "__main__":
    nc = get_nc()
    print("compiled OK")
